# revision 12
# baseline (speedup 1.0000x reference)
"""AutoCorrelation kernel for Trainium2 (Bass/Tile), 8-core data parallel.

Math: the reference computes rfft over the zero-padded head dim (D=64 -> L=512),
multiplies conj(Q)*K, irffts, then MEANS over heads AND the whole lag axis.
Summing a circular correlation over all lags factorizes:
    sum_t corr[t] = (sum_d q[d]) * (sum_d k[d])
so  x_corr_mean[b,l] = 1/(H*L) * sum_h (sum_d q[b,l,h,:]) * (sum_d k[b,l,h,:]).
Then top-6 over l per batch, softmax, weighted sum of values rows -> [B,H,D].

Sharding: batch 16 -> 2 per core across 8 cores, no cross-core communication.

Schedule (per core): batch 0 streams on the sync-engine HWDGE ring, batch 1 on
the scalar-engine ring (whose first transfer hides behind the one-time ACT
table load).  q row-sum reduces run on DVE, k row-sums + sq*sk products + the
h-reduction run on GPSIMD, so neither engine falls behind arrival.  Per chunk,
corr columns are flattened into a PSUM row via PE column-transposes (no SBUF
rake DMA); per-range partial MAX8s run as chunks complete so only a short
merge + FIND_INDEX8 remain after the last byte.  Indices hop onto partitions
via one 32x32 stream transpose and a gpsimd indirect DMA gathers the six value
rows, cast to bf16 in flight.  Softmax weights hide under the gather latency;
one bf16 matmul per batch forms the weighted sum in PSUM; ACT copies it out
and the sync ring stores it.  Batch 0's whole tail hides under batch 1's
streaming.
"""

import numpy as np

import concourse.bass as bass
import concourse.mybir as mybir
import concourse.tile as tile
from concourse.masks import make_identity
from concourse.bass_utils import run_bass_kernel_spmd

B, L, H, D = 16, 512, 8, 64
HD = H * D                  # 512
NCORES = 8
BPC = B // NCORES           # 2 batches per core
ROWS = BPC * L              # 1024 rows of [HD] per core
P = 128
TPB = L // P                # 4 chunks of 128 rows per batch
KTOP = 6                    # k = int(log(512)) = 6
SCALE = 1.0 / (H * L)

_CACHE = {}


def _emit(tc, q, k, v, out):
    # out: single [BPC, HD] DRAM AP.
    nc = tc.nc
    from contextlib import ExitStack

    with ExitStack() as ctx:
        main = ctx.enter_context(tc.tile_pool(name="main", bufs=1))
        small = ctx.enter_context(tc.tile_pool(name="small", bufs=1))
        psum = ctx.enter_context(tc.tile_pool(name="psum", bufs=1, space="PSUM"))

        ident = small.tile([P, P], mybir.dt.float32)
        make_identity(nc, ident[:])

        q3 = q.rearrange("(t p) m -> t p m", p=P)
        k3 = k.rearrange("(t p) m -> t p m", p=P)

        # q pieces stream on the sync HWDGE ring, k pieces on the scalar
        # ring; batch-0 pieces lead both rings so batch 0 completes early,
        # batch 1 trails with a short final piece for a short last reduce.
        PIECES_Q = {0: [(0, 2), (2, 4)], 1: [(0, 2), (2, 4)]}
        PIECES_K = {0: [(0, 2), (2, 4)], 1: [(0, 2), (2, 3), (3, 4)]}
        qt, kt = {}, {}
        for b in range(BPC):
            qt[b] = main.tile([P, TPB, HD], mybir.dt.float32, tag=f"qt{b}", name=f"qt{b}")
            kt[b] = main.tile([P, TPB, HD], mybir.dt.float32, tag=f"kt{b}", name=f"kt{b}")
        for b in range(BPC):
            t0 = b * TPB
            for lo, hi in PIECES_Q[b]:
                nc.sync.dma_start(
                    out=qt[b][:, lo:hi, :],
                    in_=q3[t0 + lo : t0 + hi].rearrange("t p m -> p t m"),
                )
            for lo, hi in PIECES_K[b]:
                nc.scalar.dma_start(
                    out=kt[b][:, lo:hi, :],
                    in_=k3[t0 + lo : t0 + hi].rearrange("t p m -> p t m"),
                )

        # Transpose-matmul outputs must sit at PSUM partition 0, so each
        # batch's corr row / out row gets its own bank.
        psum_corr = {
            b: psum.tile([1, L], mybir.dt.float32, tag=f"corr{b}", name=f"pcorr{b}")
            for b in range(BPC)
        }
        psum_out = {
            b: psum.tile([1, HD], mybir.dt.float32, tag=f"out{b}", name=f"pout{b}")
            for b in range(BPC)
        }

        def stream_corr(b):
            """q reduces on DVE chase the sync ring, k reduces chase the
            scalar ring; per k-piece the product + h-reduce fire, then a PE
            column-transpose per chunk into the PSUM corr row."""
            sq = small.tile([P, TPB * H], mybir.dt.float32, tag=f"sq{b}", name=f"sq{b}")
            sk = small.tile([P, TPB * H], mybir.dt.float32, tag=f"sk{b}", name=f"sk{b}")
            prod = small.tile([P, TPB * H], mybir.dt.float32, tag=f"prod{b}", name=f"prod{b}")
            corr_b = small.tile([P, TPB], mybir.dt.float32, tag=f"corr{b}", name=f"corrb{b}")
            for lo, hi in PIECES_Q[b]:
                nc.vector.reduce_sum(
                    out=sq[:, lo * H : hi * H],
                    in_=qt[b][:, lo:hi, :].rearrange("p t (h d) -> p (t h) d", d=D),
                    axis=mybir.AxisListType.X,
                )
            for lo, hi in PIECES_K[b]:
                nc.vector.reduce_sum(
                    out=sk[:, lo * H : hi * H],
                    in_=kt[b][:, lo:hi, :].rearrange("p t (h d) -> p (t h) d", d=D),
                    axis=mybir.AxisListType.X,
                )
                nc.vector.tensor_mul(
                    prod[:, lo * H : hi * H],
                    sq[:, lo * H : hi * H],
                    sk[:, lo * H : hi * H],
                )
                nc.vector.reduce_sum(
                    out=corr_b[:, lo:hi],
                    in_=prod[:, lo * H : hi * H].rearrange("p (t h) -> p t h", h=H),
                    axis=mybir.AxisListType.X,
                )
                for t in range(lo, hi):
                    nc.tensor.transpose(
                        out=psum_corr[b][0:1, t * P : (t + 1) * P],
                        in_=corr_b[:, t : t + 1],
                        identity=ident[:],
                    )
            return None

        def topk(b, maxcat):
            """MAX8 + FIND_INDEX8 straight off the PSUM corr row."""
            maxv = small.tile([1, 8], mybir.dt.float32, tag=f"maxv{b}", name=f"maxv{b}")
            nc.vector.max(out=maxv[:], in_=psum_corr[b][0:1, :])
            maxi = small.tile([1, 8], mybir.dt.uint32, tag=f"maxi{b}", name=f"maxi{b}")
            nc.vector.max_index(
                out=maxi[:], in_max=maxv[:], in_values=psum_corr[b][0:1, :]
            )
            return maxv, maxi

        # Staging tiles for the 32x32 stream transposes, zeroed early so the
        # transpose never reads uninitialized SBUF.
        stage_i = {}
        stage_w = {}
        for b in range(BPC):
            stage_i[b] = small.tile([32, 32], mybir.dt.float32, tag=f"stagei{b}", name=f"stgi{b}")
            stage_w[b] = small.tile([32, 32], mybir.dt.float32, tag=f"stagew{b}", name=f"stgw{b}")
            nc.vector.memset(stage_i[b][:], 0.0)
            nc.vector.memset(stage_w[b][:], 0.0)

        def gather(b, maxi):
            """Indices to partitions (f32 staging + 32x32 stream transpose),
            then one indirect gather of the 6 value rows, cast to bf16."""
            stage = stage_i[b]
            if b == 0:
                nc.vector.tensor_copy(stage[0:1, 0:KTOP], maxi[0:1, 0:KTOP])
            else:
                idxf = small.tile([1, 8], mybir.dt.float32, tag=f"idxf{b}", name=f"idxf{b}")
                nc.vector.tensor_copy(idxf[:], maxi[:])
                nc.vector.tensor_scalar_add(
                    stage[0:1, 0:KTOP], idxf[0:1, 0:KTOP], float(b * L)
                )
            stageT = small.tile([32, 32], mybir.dt.float32, tag=f"stageiT{b}", name=f"stageiT{b}")
            nc.vector.transpose(out=stageT[:], in_=stage[:])
            comb = small.tile([KTOP, 1], mybir.dt.uint32, tag=f"comb{b}", name=f"comb{b}")
            nc.vector.tensor_copy(comb[:], stageT[0:KTOP, 0:1])
            gath = small.tile([KTOP, HD], mybir.dt.bfloat16, tag=f"gath{b}", name=f"gath{b}")
            nc.gpsimd.indirect_dma_start(
                out=gath[:],
                out_offset=None,
                in_=v,
                in_offset=bass.IndirectOffsetOnAxis(ap=comb[:, 0:1], axis=0),
            )
            return gath

        def weights(b, maxv):
            """softmax over the top-6 of corr*SCALE (|corr*SCALE| < ~1 so the
            max-subtraction is safely skipped in fp32); weights hop onto
            partitions via one 32x32 stream transpose, cast to bf16."""
            e = small.tile([1, KTOP], mybir.dt.float32, tag=f"e{b}", name=f"e{b}")
            nc.scalar.activation(
                out=e[:],
                in_=maxv[0:1, 0:KTOP],
                func=mybir.ActivationFunctionType.Exp,
                scale=SCALE,
            )
            s = small.tile([1, 1], mybir.dt.float32, tag=f"s{b}", name=f"s{b}")
            nc.vector.reduce_sum(out=s[:], in_=e[:], axis=mybir.AxisListType.X)
            rs = small.tile([1, 1], mybir.dt.float32, tag=f"rs{b}", name=f"rs{b}")
            nc.vector.reciprocal(out=rs[:], in_=s[:])
            stage = stage_w[b]
            nc.vector.tensor_scalar_mul(stage[0:1, 0:KTOP], e[:], rs[:, 0:1])
            stageT = small.tile([32, 32], mybir.dt.float32, tag=f"stagewT{b}", name=f"stagewT{b}")
            nc.vector.transpose(out=stageT[:], in_=stage[:])
            wT = small.tile([KTOP, 1], mybir.dt.bfloat16, tag=f"wT{b}", name=f"wT{b}")
            nc.vector.tensor_copy(wT[:], stageT[0:KTOP, 0:1])
            return wT

        def finish(b, wT, gath):
            """Weighted sum (one bf16 matmul), ACT PSUM->SBUF copy, store."""
            nc.tensor.matmul(
                out=psum_out[b][0:1, :],
                lhsT=wT[:, 0:1],
                rhs=gath[:],
                start=True,
                stop=True,
            )
            outt = small.tile([1, HD], mybir.dt.float32, tag=f"outt{b}", name=f"outt{b}")
            nc.scalar.copy(outt[:], psum_out[b][0:1, :])
            nc.sync.dma_start(out=out[b : b + 1, :], in_=outt[:])

        # ---- per-batch pipelines; the tile scheduler interleaves by
        # readiness, batch 0's tail hides under batch 1's streaming. ----
        for b in range(BPC):
            maxcat = stream_corr(b)
            maxv, maxi = topk(b, maxcat)
            gath = gather(b, maxi)
            wT = weights(b, maxv)
            finish(b, wT, gath)


def _build_bass():
    import concourse.bacc as bacc

    nc = bacc.Bacc(trn_type="TRN2", target_bir_lowering=False, debug=False)
    q = nc.dram_tensor("q", [ROWS, HD], mybir.dt.float32, kind="ExternalInput").ap()
    k = nc.dram_tensor("k", [ROWS, HD], mybir.dt.float32, kind="ExternalInput").ap()
    v = nc.dram_tensor("v", [ROWS, HD], mybir.dt.float32, kind="ExternalInput").ap()
    out = nc.dram_tensor(
        "out", [BPC, HD], mybir.dt.float32, kind="ExternalOutput"
    ).ap()
    with tile.TileContext(nc) as tc:
        _emit(tc, q, k, v, out)
    nc.compile()
    return nc


def _get_nc():
    if "nc" not in _CACHE:
        _CACHE["nc"] = _build_bass()
    return _CACHE["nc"]


def run_sharded(queries, keys, values, trace=False, **kw):
    """Shard over 8 cores, run, gather. Returns (out [16,8,64], BassKernelResults)."""
    nc = _get_nc()
    q = np.ascontiguousarray(np.asarray(queries, dtype=np.float32))
    k = np.ascontiguousarray(np.asarray(keys, dtype=np.float32))
    v = np.ascontiguousarray(np.asarray(values, dtype=np.float32))
    in_maps = []
    for c in range(NCORES):
        sl = slice(c * BPC, (c + 1) * BPC)
        in_maps.append(
            {
                "q": q[sl].reshape(ROWS, HD),
                "k": k[sl].reshape(ROWS, HD),
                "v": v[sl].reshape(ROWS, HD),
            }
        )
    res = run_bass_kernel_spmd(nc, in_maps, list(range(NCORES)), trace=trace, **kw)
    out = np.empty((B, H, D), dtype=np.float32)
    for c in range(NCORES):
        out[c * BPC : (c + 1) * BPC] = res.results[c]["out"].reshape(BPC, H, D)
    return out, res


def kernel(queries, keys, values, B=None, **_ignored):
    out, _ = run_sharded(queries, keys, values, trace=False)
    return out
